# revision 1
# baseline (speedup 1.0000x reference)
"""MoE cross-attention kernel for 8 Trainium2 NeuronCores.

Problem (hardcoded): x[4,2048,256], y[4,2048,256], token_types[4,2048] int64,
Wq[256,256], Wkv[256,512], expert MLPs (s/l) with hidden 1024.

Sharding: core c -> batch b=c//2, query rows n in [1024*(c%2), +1024).
Outputs are disjoint slices, so no collectives. Host pre-transposes
activations (xT/yT) and re-assembles the output, all in numpy.

Device-side data flow per core (all matmul operands bf16, psum fp32):
  kT[256,2048], v[2048,256], qT[256,1024] projections ->
  per (head-group g of 4 heads, n-chunk of 512, m-tile of 128):
  scores^T[m,n] = k^T q via row-packed matmuls (tile_position=(32h,0),
  K=32 each), exp on ScalarE reading PSUM directly with the softmax
  scale folded into the activation's scale field (no max-subtraction:
  |scores*scale| <= ~1 for this problem), then ctx^T and the softmax
  denominator accumulate over the 16 m-tiles on the PE (col-packed
  per-head, tile_position=(0,32h)). The denominator matmul uses an
  all-ones [128,32] lhsT so its PSUM result lands already replicated
  across each head's 32 partitions -> one VectorE reciprocal + one
  multiply give normalized ctx^T with no cross-partition traffic.
  exp tiles are pre-summed in bf16 pairs-of-pairs on the otherwise-idle
  VectorE, so the denominator matmul only runs on every 4th m-tile
  (4x fewer PE ops for the softmax row-sums).
  Then both expert MLPs (gelu+bias on ScalarE, FD=1024) and the
  token_type select on VectorE; out^T DMAs back and the host
  transposes/reassembles.

Softmax exp is the critical path: 134M elements / 8 cores / 128 lanes
/ 1.2 GHz ~= 109 us of ScalarE time per core; everything else is
structured to overlap under it.
"""

import os
from contextlib import ExitStack

import numpy as np
import ml_dtypes

import concourse.bass as bass
import concourse.mybir as mybir
import concourse.tile as tile
from concourse import bacc
from concourse.bass_utils import run_bass_kernel_spmd

NCORES = 8
B, N, M, C = 4, 2048, 2048, 256
H, D, HD = 8, 32, 1024
NT = N // 2  # query tokens per core
SCALE = float(D) ** -0.5

F32 = mybir.dt.float32
BF16 = mybir.dt.bfloat16
AF = mybir.ActivationFunctionType

_CACHED_NC = None
_last_in_maps = None


def _build(reps=1, phases='ABC', SP_BUFS=3, CD_BUFS=1, EP_BUFS=4, PA_BUFS=2, MP_BUFS=2):
    nc = bacc.Bacc("TRN2", target_bir_lowering=False, debug=False,
                   num_devices=NCORES)

    # ---- DRAM I/O ----
    xT = nc.dram_tensor("xT", [C, NT], BF16, kind="ExternalInput").ap()
    yT = nc.dram_tensor("yT", [C, M], BF16, kind="ExternalInput").ap()
    wq = nc.dram_tensor("wq", [C, C], BF16, kind="ExternalInput").ap()
    wkv = nc.dram_tensor("wkv", [C, 2 * C], BF16, kind="ExternalInput").ap()
    w1s = nc.dram_tensor("w1s", [C, HD], BF16, kind="ExternalInput").ap()
    w1l = nc.dram_tensor("w1l", [C, HD], BF16, kind="ExternalInput").ap()
    w2s = nc.dram_tensor("w2s", [HD, C], BF16, kind="ExternalInput").ap()
    w2l = nc.dram_tensor("w2l", [HD, C], BF16, kind="ExternalInput").ap()
    b1s = nc.dram_tensor("b1s", [128, HD // 128], F32, kind="ExternalInput").ap()
    b1l = nc.dram_tensor("b1l", [128, HD // 128], F32, kind="ExternalInput").ap()
    b2s = nc.dram_tensor("b2s", [128, C // 128], F32, kind="ExternalInput").ap()
    b2l = nc.dram_tensor("b2l", [128, C // 128], F32, kind="ExternalInput").ap()
    msk = nc.dram_tensor("msk", [128, NT], F32, kind="ExternalInput").ap()
    ones32 = nc.dram_tensor("ones32", [128, 32], BF16, kind="ExternalInput").ap()
    outT = nc.dram_tensor("outT", [C, NT], F32, kind="ExternalOutput").ap()

    with tile.TileContext(nc) as tc, ExitStack() as ctx:
        cp = ctx.enter_context(tc.tile_pool(name="consts", bufs=1))

        def load(shape, dtype, src, tag):
            t = cp.tile(shape, dtype, tag=tag, name=tag)
            nc.sync.dma_start(t[:], src)
            return t

        # persistent inputs (partition-tiled by rows of the DRAM tensor).
        # Emission order = DMA priority: the kv projection consumes
        # wkv/yT first, so load those before everything else.
        ones_t = load([128, 32], BF16, ones32[:], "ones32")
        wkv_t = [load([128, 2 * C], BF16, wkv[bass.ts(k, 128), :], f"wkv{k}") for k in range(2)]
        yT_t = [load([128, M], BF16, yT[bass.ts(k, 128), :], f"yT{k}") for k in range(2)]
        wq_t = [load([128, C], BF16, wq[bass.ts(k, 128), :], f"wq{k}") for k in range(2)]
        xT_t = [load([128, NT], BF16, xT[bass.ts(k, 128), :], f"xT{k}") for k in range(2)]
        w1_t = {e: [load([128, HD], BF16, w[bass.ts(k, 128), :], f"w1{e}{k}")
                    for k in range(2)]
                for e, w in (("s", w1s), ("l", w1l))}
        w2_t = {e: [load([128, C], BF16, w[bass.ts(k, 128), :], f"w2{e}{k}")
                    for k in range(8)]
                for e, w in (("s", w2s), ("l", w2l))}
        b1_t = {e: load([128, HD // 128], F32, b[:], f"b1{e}")
                for e, b in (("s", b1s), ("l", b1l))}
        b2_t = {e: load([128, C // 128], F32, b[:], f"b2{e}")
                for e, b in (("s", b2s), ("l", b2l))}
        msk_t = load([128, NT], F32, msk[:], "msk")

        # Preload the exp ACT table while ScalarE is idle at kernel start:
        # a dummy 1-element Exp pulls PSEUDO_LOAD_ACT_FUNC_SET off the
        # critical path (saves ~2.7us before the first real exp).
        warm_t = cp.tile([1, 1], F32, tag="warm", name="warm")
        nc.scalar.activation(warm_t[:], ones_t[0:1, 0:1], AF.Exp)

        # persistent activations
        kT_t = [cp.tile([128, M], BF16, tag=f"kT{g}", name=f"kT{g}") for g in range(2)]
        v_t = [cp.tile([128, C], BF16, tag=f"v{mt}", name=f"v{mt}") for mt in range(16)]
        qT_t = [cp.tile([128, NT], BF16, tag=f"qT{g}", name=f"qT{g}") for g in range(2)]
        ctxT_t = [cp.tile([128, NT], BF16, tag=f"ctxT{g}", name=f"ctxT{g}") for g in range(2)]

        for _rep in range(reps):
            # ---- Phase A: projections ----
            with tc.tile_pool(name="pA", bufs=PA_BUFS, space="PSUM") as pA:
              if 'A' in phases:
                for g in range(2):
                    # kT for group g
                    ps = pA.tile([128, M], F32, tag="pa")
                    for mc in range(M // 512):
                        for k in range(2):
                            nc.tensor.matmul(ps[:, bass.ts(mc, 512)],
                                             wkv_t[k][:, bass.ts(g, 128)],
                                             yT_t[k][:, bass.ts(mc, 512)],
                                             start=(k == 0), stop=(k == 1))
                        nc.vector.tensor_copy(kT_t[g][:, bass.ts(mc, 512)],
                                              ps[:, bass.ts(mc, 512)])
                    # qT for group g right away so attention(g) can start
                    ps = pA.tile([128, NT], F32, tag="pa")
                    for nc_ in range(NT // 512):
                        for k in range(2):
                            nc.tensor.matmul(ps[:, bass.ts(nc_, 512)],
                                             wq_t[k][:, bass.ts(g, 128)],
                                             xT_t[k][:, bass.ts(nc_, 512)],
                                             start=(k == 0), stop=(k == 1))
                        nc.vector.tensor_copy(qT_t[g][:, bass.ts(nc_, 512)],
                                              ps[:, bass.ts(nc_, 512)])
                for mt in range(16):
                    ps = pA.tile([128, C], F32, tag="pa")
                    for k in range(2):
                        nc.tensor.matmul(ps[:], yT_t[k][:, bass.ts(mt, 128)],
                                         wkv_t[k][:, C:2 * C],
                                         start=(k == 0), stop=(k == 1))
                    nc.vector.tensor_copy(v_t[mt][:], ps[:])

            # ---- Phase B: attention ----
            with tc.tile_pool(name="sP", bufs=SP_BUFS, space="PSUM") as sP, \
                 tc.tile_pool(name="cP", bufs=CD_BUFS, space="PSUM") as cP, \
                 tc.tile_pool(name="dP", bufs=CD_BUFS, space="PSUM") as dP, \
                 tc.tile_pool(name="eP", bufs=EP_BUFS) as eP, \
                 tc.tile_pool(name="rP", bufs=2) as rP:
              if 'B' in phases:
                for g in range(2):
                    for nc_ in range(NT // 512):
                        ctx_ps = cP.tile([128, 512], F32, tag="ctx")
                        den_ps = dP.tile([128, 512], F32, tag="den")
                        qsums = []
                        for pair in range(8):
                            pexp = []
                            for sub in range(2):
                                mt = 2 * pair + sub
                                exp_sb = eP.tile([128, 2048], BF16, tag="exp",
                                                 name=f"exp{sub}")
                                for half in range(2):
                                    s_ps = sP.tile([128, 1024], F32, tag="s")
                                    for hh in range(2):
                                        h = 2 * half + hh
                                        nc.tensor.matmul(
                                            s_ps[:, bass.ts(hh, 512)],
                                            kT_t[g][bass.ts(h, 32), bass.ts(mt, 128)],
                                            qT_t[g][bass.ts(h, 32), bass.ts(nc_, 512)],
                                            start=True, stop=True,
                                            tile_position=(32 * h, 0))
                                    nc.scalar.activation(
                                        exp_sb[:, bass.ts(half, 1024)], s_ps[:],
                                        AF.Exp, scale=SCALE)
                                for h in range(4):
                                    nc.tensor.matmul(
                                        ctx_ps[bass.ts(h, 32), :],
                                        v_t[mt][:, bass.ts(4 * g + h, 32)],
                                        exp_sb[:, bass.ts(h, 512)],
                                        start=(mt == 0), stop=(mt == 15),
                                        tile_position=(0, 32 * h))
                                pexp.append(exp_sb)
                            # pair-sum on VectorE (bf16 2x); second level sums
                            # pairs-of-pairs so den matmuls drop 16 -> 4 per chunk
                            sum_sb = eP.tile([128, 2048], BF16, tag="esum")
                            nc.vector.tensor_add(sum_sb[:], pexp[0][:], pexp[1][:])
                            qsums.append(sum_sb)
                            if pair % 2 == 1:
                                q_sb = eP.tile([128, 2048], BF16, tag="eqsum")
                                nc.vector.tensor_add(q_sb[:], qsums[-2][:],
                                                     qsums[-1][:])
                                qsums.append(q_sb)
                            if pair % 4 == 3:
                                o_sb = eP.tile([128, 2048], BF16, tag="eosum")
                                nc.vector.tensor_add(o_sb[:], qsums[-4][:],
                                                     qsums[-1][:])
                                for h in range(4):
                                    nc.tensor.matmul(
                                        den_ps[bass.ts(h, 32), :],
                                        ones_t[:],
                                        o_sb[:, bass.ts(h, 512)],
                                        start=(pair == 3), stop=(pair == 7),
                                        tile_position=(0, 32 * h))
                        recip_sb = rP.tile([128, 512], F32, tag="recip")
                        nc.vector.reciprocal(recip_sb[:], den_ps[:])
                        nc.vector.tensor_mul(ctxT_t[g][:, bass.ts(nc_, 512)],
                                             ctx_ps[:], recip_sb[:])

            # ---- Phase C: MLP experts + select ----
            hT_t = {e: [cp.tile([128, NT], BF16, tag=f"hT{e}{p}", name=f"hT{e}{p}") for p in range(8)]
                    for e in ("s", "l")}
            with tc.tile_pool(name="mP", bufs=MP_BUFS, space="PSUM") as mP, \
                 tc.tile_pool(name="m2P", bufs=4, space="PSUM") as m2P, \
                 tc.tile_pool(name="oP", bufs=6) as oP:
              if 'C' in phases:
                for e in ("s", "l"):
                    for p in range(8):
                        ps = mP.tile([128, NT], F32, tag="mh")
                        for nc_ in range(NT // 512):
                            for k in range(2):
                                nc.tensor.matmul(ps[:, bass.ts(nc_, 512)],
                                                 w1_t[e][k][:, bass.ts(p, 128)],
                                                 ctxT_t[k][:, bass.ts(nc_, 512)],
                                                 start=(k == 0), stop=(k == 1))
                        nc.scalar.activation(hT_t[e][p][:], ps[:], AF.Gelu,
                                             bias=b1_t[e][:, p:p + 1], scale=1.0)
                outT_sb = [cp.tile([128, NT], F32, tag=f"oT{pt}", name=f"oT{pt}") for pt in range(2)]
                for nc_ in range(NT // 512):
                    for pt in range(2):
                        ps_s = m2P.tile([128, 512], F32, tag="mm")
                        for k in range(8):
                            nc.tensor.matmul(ps_s[:],
                                             w2_t["s"][k][:, bass.ts(pt, 128)],
                                             hT_t["s"][k][:, bass.ts(nc_, 512)],
                                             start=(k == 0), stop=(k == 7))
                        os_sb = oP.tile([128, 512], F32, tag="os")
                        nc.vector.tensor_scalar_add(os_sb[:], ps_s[:],
                                                    b2_t["s"][:, pt:pt + 1])
                        ps_l = m2P.tile([128, 512], F32, tag="mm")
                        for k in range(8):
                            nc.tensor.matmul(ps_l[:],
                                             w2_t["l"][k][:, bass.ts(pt, 128)],
                                             hT_t["l"][k][:, bass.ts(nc_, 512)],
                                             start=(k == 0), stop=(k == 7))
                        ol_sb = oP.tile([128, 512], F32, tag="ol")
                        nc.vector.tensor_scalar_add(ol_sb[:], ps_l[:],
                                                    b2_t["l"][:, pt:pt + 1])
                        df_sb = oP.tile([128, 512], F32, tag="df")
                        nc.vector.tensor_sub(df_sb[:], ol_sb[:], os_sb[:])
                        pr_sb = oP.tile([128, 512], F32, tag="pr")
                        nc.vector.tensor_mul(pr_sb[:], df_sb[:],
                                             msk_t[:, bass.ts(nc_, 512)])
                        nc.vector.tensor_add(outT_sb[pt][:, bass.ts(nc_, 512)],
                                             os_sb[:], pr_sb[:])
                        nc.sync.dma_start(
                            outT[bass.ts(pt, 128), bass.ts(nc_, 512)],
                            outT_sb[pt][:, bass.ts(nc_, 512)])

    nc.compile()
    return nc


def _get_nc():
    global _CACHED_NC
    if _CACHED_NC is None:
        _CACHED_NC = _build()
    return _CACHED_NC


def kernel(x, y, token_types, Wq, Wkv, Ws1, bs1, Ws2, bs2, Wl1, bl1, Wl2, bl2):
    x = np.asarray(x, dtype=np.float32)
    y = np.asarray(y, dtype=np.float32)
    tt = np.asarray(token_types)

    bf = lambda a: np.ascontiguousarray(np.asarray(a, np.float32)).astype(ml_dtypes.bfloat16)
    f32 = lambda a: np.ascontiguousarray(np.asarray(a, np.float32))

    shared = {
        "wq": bf(Wq), "wkv": bf(Wkv),
        "w1s": bf(Ws1), "w1l": bf(Wl1), "w2s": bf(Ws2), "w2l": bf(Wl2),
        # bias b[1024] -> [128, 8] with sbuf tile p = cols: b_r[c, p] = b[128p + c]
        "b1s": f32(np.asarray(bs1, np.float32).reshape(8, 128).T),
        "b1l": f32(np.asarray(bl1, np.float32).reshape(8, 128).T),
        "b2s": f32(np.asarray(bs2, np.float32).reshape(2, 128).T),
        "b2l": f32(np.asarray(bl2, np.float32).reshape(2, 128).T),
        "ones32": np.ones((128, 32), ml_dtypes.bfloat16),
    }
    in_maps = []
    for c in range(NCORES):
        b, half = divmod(c, 2)
        n0 = half * NT
        m = np.broadcast_to(
            tt[b, n0:n0 + NT].astype(np.float32)[None, :], (128, NT))
        in_maps.append({
            **shared,
            "xT": bf(x[b, n0:n0 + NT, :].T),
            "yT": bf(y[b].T),
            "msk": np.ascontiguousarray(m),
        })

    global _last_in_maps
    _last_in_maps = in_maps
    nc = _get_nc()
    res = run_bass_kernel_spmd(nc, in_maps, core_ids=list(range(NCORES)))

    out = np.empty((B, N, C), dtype=np.float32)
    for c in range(NCORES):
        b, half = divmod(c, 2)
        n0 = half * NT
        out[b, n0:n0 + NT, :] = res.results[c]["outT"].T
    return out

